# revision 17
# baseline (speedup 1.0000x reference)
"""Multi-head attention (B=2, S=2048, D=1024, H=16) on 8 NeuronCores.

Sharding: core c -> (batch b = c // 4, head-group g = c % 4). Each core
computes 4 heads of one batch plus the partial output projection for its
head-group's rows of Wo; the host sums the 4 partials per batch and adds bo.

Key-side compaction: masked key positions are dropped on the host; the
compacted length is padded to a multiple of 128 with zero columns whose
exp-bias (-60) keeps their softmax weight at ~0.

v2 structural change vs v1: single fused emission stream. The projection
matmuls (Q chunks 1-3), V-projection, output-projection m-tiles and the
normalization broadcasts are interleaved INTO the attention unit loop as PE
filler, so the Tensor engine streams continuously while the ACT engine
(which carries the exp() pass, ~1.05us per [128,1024] score tile — the
serial floor of the kernel) stays saturated from ~15us onward instead of
idling through two serial projection phases.

Engine placement: exp + K/Q0 bias-evacs on ACT; V/Q-filler/ctx-norm/output
evacuations on DVE; reciprocal via reciprocal_approx_fast (fp32 bit trick,
~5x the iterative divide) + a 1-row f16 cast; normalization broadcast runs
as f16 K=1 matmuls (f32r moving operands run the 4-pass fp32 HIGH mode on
real HW). Output partials leave in bf16 on the GpSimd DMA queue.
"""

import os
from collections import defaultdict
from contextlib import ExitStack

import numpy as np

import concourse.bacc as bacc
import concourse.mybir as mybir
import concourse.tile as tile

F32 = mybir.dt.float32
F32R = mybir.dt.float32r
BF16 = mybir.dt.bfloat16
F16 = mybir.dt.float16
FP8 = mybir.dt.float8e4
AF = mybir.ActivationFunctionType

B, S, D = 2, 2048, 1024
H, DK = 16, 64
G = 4                    # head-groups (tensor parallel)
HPG = H // G             # 4 heads per group
DG = HPG * DK            # 256 head dims per group
NCORES = 8
MASK_NEG = -60.0         # additive post-scale bias for padded key positions
SCALE = 0.125            # 1/sqrt(dk)

KT_D = D // 128          # 8 contraction tiles for projections
NT = DG // 128           # 2 partition-tiles of qT/kT/cT (one head-pair each)
QC = 512                 # q chunk (matmul moving dim)
NQC = S // QC            # 4
VW = HPG * (DK + 1)      # 260: V width incl. per-head ones column

KDT = os.environ.get("KDT", "bf16")


def _dt():
    return BF16 if KDT == "bf16" else F32R


def _np_dt():
    import ml_dtypes

    return ml_dtypes.bfloat16 if KDT == "bf16" else np.float32


def build_bass(ktk):
    """Build the SPMD program for `ktk` 128-wide key tiles (SK = 128*ktk)."""
    SK = 128 * ktk
    kchunks = [(n0, min(QC, SK - n0)) for n0 in range(0, SK, QC)]
    cdt = _dt()
    NU = NQC * NT        # 8 attention units

    nc = bacc.Bacc(None, target_bir_lowering=False, debug=False)

    xq = nc.dram_tensor("xq", [D, S], cdt, kind="ExternalInput")
    xk = nc.dram_tensor("xk", [D, SK], cdt, kind="ExternalInput")
    xv = nc.dram_tensor("xv", [D, SK], cdt, kind="ExternalInput")
    wq = nc.dram_tensor("wq", [D, DG], cdt, kind="ExternalInput")
    wk = nc.dram_tensor("wk", [D, DG], cdt, kind="ExternalInput")
    wv = nc.dram_tensor("wv", [D + 1, VW], cdt, kind="ExternalInput")
    wo = nc.dram_tensor("wo", [DG, D], cdt, kind="ExternalInput")
    bq = nc.dram_tensor("bq", [128, NT], F32, kind="ExternalInput")
    bk = nc.dram_tensor("bk", [128, NT], F32, kind="ExternalInput")
    mb = nc.dram_tensor("mb", [128, ktk], F32, kind="ExternalInput")
    cst = nc.dram_tensor("cst", [3, 128], F16, kind="ExternalInput")
    cstc = nc.dram_tensor("cstc", [1, 128], cdt, kind="ExternalInput")
    out = nc.dram_tensor("out", [S, D], BF16, kind="ExternalOutput")

    with tile.TileContext(nc) as tc, ExitStack() as ctx:
        consts = ctx.enter_context(tc.tile_pool(name="consts", bufs=1))
        resid = ctx.enter_context(tc.tile_pool(name="resid", bufs=1))
        stream = ctx.enter_context(tc.tile_pool(name="stream", bufs=1))
        ptp = ctx.enter_context(tc.tile_pool(name="ptp", bufs=6))
        smalls = ctx.enter_context(tc.tile_pool(name="smalls", bufs=3))
        obp = ctx.enter_context(tc.tile_pool(name="obp", bufs=3))

        # ---------------- weights / consts / input streams ----------------
        # DMA emission order == consumption order; everything rides the Sync
        # HWDGE queue except output tiles (GpSimd queue).
        wk_s = consts.tile([128, KT_D, DG], cdt, tag="wk_s", name="wk_s")
        nc.sync.dma_start(out=wk_s, in_=wk[:].rearrange("(kt p) n -> p kt n", p=128))
        bk_s = consts.tile([128, NT], F32, tag="bk_s", name="bk_s")
        nc.gpsimd.dma_start(out=bk_s, in_=bk[:])
        mb_s = consts.tile([128, ktk], F32, tag="mb_s", name="mb_s")
        nc.gpsimd.dma_start(out=mb_s, in_=mb[:])
        ones1 = consts.tile([1, 128], cdt, tag="ones1", name="ones1")
        nc.gpsimd.dma_start(out=ones1, in_=cstc[0:1, :])
        onesA = consts.tile([1, 128], F16, tag="onesA", name="onesA")
        nc.gpsimd.dma_start(out=onesA, in_=cst[1:2, :])
        onesB = consts.tile([1, 128], F16, tag="onesB", name="onesB")
        nc.gpsimd.dma_start(out=onesB, in_=cst[2:3, :])

        # xk in 3 kt-segments so K-projection can chase the DMA
        xk_t = stream.tile([128, KT_D, SK], cdt, tag="xk", name="xk_t")
        for s0 in range(0, KT_D, 3):
            s1 = min(s0 + 3, KT_D)
            nc.sync.dma_start(
                out=xk_t[:, s0:s1, :],
                in_=xk[s0 * 128 : s1 * 128, :].rearrange(
                    "(kt p) n -> p kt n", p=128
                ),
            )

        # wq + xq0 follow xk on the Sync queue (consumption order); the less
        # critical wv/xq1/wo ride the Scalar HWDGE queue in parallel.
        wq_s = consts.tile([128, KT_D, DG], cdt, tag="wq_s", name="wq_s")
        nc.sync.dma_start(out=wq_s, in_=wq[:].rearrange("(kt p) n -> p kt n", p=128))
        bq_s = consts.tile([128, NT], F32, tag="bq_s", name="bq_s")
        nc.gpsimd.dma_start(out=bq_s, in_=bq[:])

        xq_tiles = []

        def load_xq(qc, eng=None):
            t_ = stream.tile(
                [128, KT_D, QC], cdt, tag="xq", bufs=3, name=f"xq{qc}"
            )
            (eng or nc.sync).dma_start(
                out=t_,
                in_=xq[:, qc * QC : (qc + 1) * QC].rearrange(
                    "(kt p) c -> p kt c", p=128
                ),
            )
            xq_tiles.append(t_)

        load_xq(0)

        wv_s = consts.tile([128, KT_D, VW], cdt, tag="wv_s", name="wv_s")
        nc.scalar.dma_start(
            out=wv_s, in_=wv[0:D, :].rearrange("(kt p) n -> p kt n", p=128)
        )
        wv_b = consts.tile([1, VW], cdt, tag="wv_b", name="wv_b")
        nc.gpsimd.dma_start(out=wv_b, in_=wv[D : D + 1, :])

        xv_t = stream.tile([128, KT_D, SK], cdt, tag="xv", name="xv_t")

        def load_xv(m):
            nc.sync.dma_start(
                out=xv_t[:, :, m * 128 : (m + 1) * 128],
                in_=xv[:, m * 128 : (m + 1) * 128].rearrange(
                    "(kt p) c -> p kt c", p=128
                ),
            )

        load_xq(1, eng=nc.scalar)
        for m in range(ktk):
            load_xv(m)
        wo_s = consts.tile([128, NT, D], cdt, tag="wo_s", name="wo_s")
        nc.scalar.dma_start(
            out=wo_s, in_=wo[:].rearrange("(t p) n -> p t n", p=128)
        )
        load_xq(2)
        load_xq(3)

        # ---------------- resident activations ----------------
        qT = [resid.tile([128, S], cdt, tag=f"qT{t}", name=f"qT{t}") for t in range(NT)]
        kT = [resid.tile([128, SK], cdt, tag=f"kT{t}", name=f"kT{t}") for t in range(NT)]
        v_s = resid.tile([128, ktk, VW], cdt, tag="v_s", name="v_s")
        cT = [resid.tile([128, S], cdt, tag=f"cT{t}", name=f"cT{t}") for t in range(NT)]

        with tc.tile_pool(name="pa", bufs=1, space="PSUM") as pa:
            # PSUM budget: tags ps([128,1024]f32, 2 banks) x2 bufs + pcA/pcB
            # ([65,512]f32, 1 bank) x2 bufs each = 8 banks. Projections,
            # pbc and pom all borrow slots from the "ps" rotation.
            def kproj_chunk(t, ci, on_act=True):
                n0, w = kchunks[ci]
                psk = pa.tile([128, QC], F32, tag="ps", bufs=2, name="psk")
                for kt in range(KT_D):
                    nc.tensor.matmul(
                        psk[:, 0:w],
                        lhsT=wk_s[:, kt, t * 128 : (t + 1) * 128],
                        rhs=xk_t[:, kt, n0 : n0 + w],
                        start=(kt == 0),
                        stop=(kt == KT_D - 1),
                    )
                if on_act:
                    nc.scalar.activation(
                        out=kT[t][:, n0 : n0 + w],
                        in_=psk[:, 0:w],
                        func=AF.Identity,
                        bias=bk_s[:, t : t + 1],
                        scale=1.0,
                    )
                else:
                    nc.vector.tensor_scalar_add(
                        kT[t][:, n0 : n0 + w], psk[:, 0:w], bk_s[:, t : t + 1]
                    )

            def qproj(qc, t, on_act=False):
                xt = xq_tiles[qc]
                psq = pa.tile([128, QC], F32, tag="ps", bufs=2, name="psq")
                for kt in range(KT_D):
                    nc.tensor.matmul(
                        psq[:],
                        lhsT=wq_s[:, kt, t * 128 : (t + 1) * 128],
                        rhs=xt[:, kt, :],
                        start=(kt == 0),
                        stop=(kt == KT_D - 1),
                    )
                qsl = slice(qc * QC, (qc + 1) * QC)
                if on_act:
                    nc.scalar.activation(
                        out=qT[t][:, qsl],
                        in_=psq[:],
                        func=AF.Identity,
                        bias=bq_s[:, t : t + 1],
                        scale=1.0,
                    )
                else:
                    nc.vector.tensor_scalar_add(
                        qT[t][:, qsl], psq[:], bq_s[:, t : t + 1]
                    )

            def vproj(m):
                pvm = pa.tile([128, VW], F32, tag="ps", bufs=2, name="pvm")
                for kt in range(KT_D):
                    nc.tensor.matmul(
                        pvm[:],
                        lhsT=xv_t[:, kt, m * 128 : (m + 1) * 128],
                        rhs=wv_s[:, kt, :],
                        start=(kt == 0),
                        stop=False,
                    )
                nc.tensor.matmul(
                    pvm[:], lhsT=ones1[:], rhs=wv_b[:], start=False, stop=True
                )
                nc.vector.tensor_copy(v_s[:, m, :], pvm[:])

            def emit_final_m(m, on_act=False):
                pom = pa.tile([128, D], F32, tag="ps", bufs=2, name="pom")
                for oc in range(2):
                    for t in range(NT):
                        nc.tensor.matmul(
                            pom[:, oc * 512 : (oc + 1) * 512],
                            lhsT=cT[t][:, m * 128 : (m + 1) * 128],
                            rhs=wo_s[:, t, oc * 512 : (oc + 1) * 512],
                            start=(t == 0),
                            stop=(t == NT - 1),
                        )
                ob = obp.tile([128, D], BF16, tag="ob", name="ob")
                if on_act:
                    nc.scalar.copy(ob[:], pom[:])
                else:
                    nc.vector.tensor_copy(ob[:], pom[:])
                nc.gpsimd.dma_start(out=out[m * 128 : (m + 1) * 128, :], in_=ob[:])

            def emit_recips(pcA, pcB, act_stage=False):
                # reciprocal_approx_fast only works at partition 0, so the
                # denominator rows (row 64 of the ctx PSUMs) are staged there
                # first; the f16 cast feeds the full-rate PE broadcast. The
                # last unit stages via ACT (idle after the final exp) so the
                # tail chain doesn't queue behind DVE evacuations.
                den = smalls.tile([1, 2 * QC], F32, tag="den", name="den")
                if act_stage:
                    nc.scalar.copy(den[0:1, 0:QC], pcA[64:65, :])
                    nc.scalar.copy(den[0:1, QC : 2 * QC], pcB[64:65, :])
                else:
                    nc.vector.tensor_copy(den[0:1, 0:QC], pcA[64:65, :])
                    nc.vector.tensor_copy(den[0:1, QC : 2 * QC], pcB[64:65, :])
                rec32 = smalls.tile([1, 2 * QC], F32, tag="rec32", name="rec32")
                nc.vector.reciprocal_approx_fast(rec32[:], den[:])
                rec = smalls.tile([1, 2 * QC], F16, tag="rec", name="rec")
                with nc.allow_low_precision(reason="f16 feed for PE broadcast"):
                    nc.vector.tensor_copy(rec[:], rec32[:])
                return rec

            def emit_norm(qc, p, pcA, pcB, rec):
                qsl = slice(qc * QC, (qc + 1) * QC)
                pbc = pa.tile([128, QC], F32, tag="ps", bufs=2, name="pbc")
                nc.tensor.matmul(
                    pbc[:],
                    lhsT=onesA[:],
                    rhs=rec[0:1, 0:QC],
                    start=True,
                    stop=False,
                )
                nc.tensor.matmul(
                    pbc[:],
                    lhsT=onesB[:],
                    rhs=rec[0:1, QC : 2 * QC],
                    start=False,
                    stop=True,
                )
                bcs = smalls.tile([128, QC], F32, tag="bcs", name="bcs")
                nc.vector.tensor_copy(bcs[:], pbc[:])
                nc.vector.tensor_mul(cT[p][0:64, qsl], pcA[0:64, :], bcs[0:64, :])
                nc.vector.tensor_mul(
                    cT[p][64:128, qsl], pcB[0:64, :], bcs[64:128, :]
                )

            # ---------------- prologue ----------------
            # kt-major K-proj for pair 0: consumes xk segments as they land
            # instead of waiting for the full tensor like the chunk-major
            # filler form does.
            PKTAGS = ["pcA", "pcB", "ps", "ps"]
            pk = [
                pa.tile([128, QC], F32, tag=PKTAGS[ci], bufs=2, name=f"pk{ci}")
                for ci in range(len(kchunks))
            ]
            for kt in range(KT_D):
                for ci, (n0, w) in enumerate(kchunks):
                    nc.tensor.matmul(
                        pk[ci][:, 0:w],
                        lhsT=wk_s[:, kt, 0:128],
                        rhs=xk_t[:, kt, n0 : n0 + w],
                        start=(kt == 0),
                        stop=(kt == KT_D - 1),
                    )
            for ci, (n0, w) in enumerate(kchunks):
                nc.scalar.activation(
                    out=kT[0][:, n0 : n0 + w],
                    in_=pk[ci][:, 0:w],
                    func=AF.Identity,
                    bias=bk_s[:, 0:1],
                    scale=1.0,
                )
            qproj(0, 0, on_act=True)

            # ---------------- fused attention + filler schedule ----------
            units = [(qc, p) for qc in range(NQC) for p in range(NT)]
            unit_fillers = defaultdict(list)
            tail_fillers = []
            # Unit 0 carries K-pair-1 and Q(0,1); its V-projection m-tiles
            # ride inline one iteration ahead of the consuming ctx matmuls.
            nkc = len(kchunks)
            for ci in range(nkc):
                unit_fillers[0].append(
                    lambda ci=ci: kproj_chunk(1, ci, on_act=False)
                )
            unit_fillers[0].append(lambda: qproj(0, 1))
            # Q projections for chunks 1..3, at least one unit ahead.
            qlist = [(qc, t) for qc in range(1, NQC) for t in range(NT)]
            slots = [1, 1, 2, 3, 4, 5]
            for (qc, t), u in zip(qlist, slots):
                unit_fillers[u].append(lambda qc=qc, t=t: qproj(qc, t))
            # Output projection m-tiles: legal from unit 2qc+2 (norm of the
            # second pair of qc is emitted at that unit's kt==2).
            for qc in range(NQC):
                base = 2 * qc + 2
                for j in range(4):
                    m = qc * 4 + j
                    u = base + j // 2 + (1 if qc < 2 else 0)
                    if u < NU and qc < 3 and m != 11:
                        unit_fillers[u].append(
                            lambda m=m, a=(u >= 6): emit_final_m(m, on_act=a)
                        )

            norm_kt = min(2, ktk - 1)
            pend = {}
            normed = set()
            state = {}
            seq = [(i, kt) for i in range(NU) for kt in range(ktk)]
            fqs = {i: list(unit_fillers.get(i, ())) for i in range(NU)}

            def emit_scores(i, kt):
                qc, p = units[i]
                qsl = slice(qc * QC, (qc + 1) * QC)
                ksl = slice(kt * 128, (kt + 1) * 128)
                ps = pa.tile([128, 2 * QC], F32, tag="ps", bufs=2, name="ps")
                nc.tensor.matmul(
                    ps[:, 0:QC],
                    lhsT=kT[p][0:64, ksl],
                    rhs=qT[p][0:64, qsl],
                    start=True,
                    stop=True,
                )
                nc.tensor.matmul(
                    ps[:, QC : 2 * QC],
                    lhsT=kT[p][64:128, ksl],
                    rhs=qT[p][64:128, qsl],
                    start=True,
                    stop=True,
                )
                pt = ptp.tile([128, 2 * QC], cdt, tag="pt", name="pt")
                nc.scalar.activation(
                    out=pt[:],
                    in_=ps[:],
                    func=AF.Exp,
                    bias=mb_s[:, kt : kt + 1],
                    scale=SCALE,
                )
                state[i]["pts"].append(pt)

            def emit_ctx(i, ct):
                qc, p = units[i]
                st = state[i]
                if ct == 0:
                    st["pcA"] = pa.tile([65, QC], F32, tag="pcA", bufs=2, name="pcA")
                    st["pcB"] = pa.tile([65, QC], F32, tag="pcB", bufs=2, name="pcB")
                hA, hB = 2 * p, 2 * p + 1
                ptc = st["pts"][ct]
                nc.tensor.matmul(
                    st["pcA"][:],
                    lhsT=v_s[:, ct, hA * 65 : (hA + 1) * 65],
                    rhs=ptc[:, 0:QC],
                    start=(ct == 0),
                    stop=(ct == ktk - 1),
                )
                nc.tensor.matmul(
                    st["pcB"][:],
                    lhsT=v_s[:, ct, hB * 65 : (hB + 1) * 65],
                    rhs=ptc[:, QC : 2 * QC],
                    start=(ct == 0),
                    stop=(ct == ktk - 1),
                )
                if ct == ktk - 1:
                    pend[i] = (
                        st["pcA"],
                        st["pcB"],
                        emit_recips(st["pcA"], st["pcB"], act_stage=(i == NU - 1)),
                    )

            for g, (i, kt) in enumerate(seq):
                if kt == 0:
                    state[i] = {"pts": []}
                emit_scores(i, kt)
                fill_kts = (1, 3, 5, 7) if i == 0 else (3, 5, 7)
                if kt in fill_kts and fqs[i]:
                    fqs[i].pop(0)()
                if kt == ktk - 1:
                    while fqs[i]:  # tiny-ktk fallback: drain before unit ends
                        fqs[i].pop(0)()
                if i == 0 and kt < ktk:
                    vproj(kt)
                if g >= 1:
                    pi, pkt = seq[g - 1]
                    emit_ctx(pi, pkt)
                if i >= 1 and i - 1 in pend and i - 1 not in normed and kt >= norm_kt:
                    normed.add(i - 1)
                    pqc, pp_ = units[i - 1]
                    emit_norm(pqc, pp_, *pend[i - 1])
            emit_ctx(*seq[-1])

            lqc, lp = units[-1]
            emit_final_m(11)  # reserved bridge tile: keeps the PE warm
            emit_norm(lqc, lp, *pend[NU - 1])
            for qc3 in range(4):
                emit_final_m(12 + qc3, on_act=True)

    nc.compile()
    return nc


def _const_rows():
    cst = np.zeros((3, 128), np.float16)
    cst[0, :] = 1.0
    cst[1, 0:64] = 1.0
    cst[2, 64:128] = 1.0
    return cst


def make_in_maps(query, key, value, mask, Wq, bq, Wk, bk, Wv, bv, Wo, bo):
    """Returns (in_maps, ktk). Key positions with mask=True are dropped."""
    query = np.asarray(query, np.float32)
    key = np.asarray(key, np.float32)
    value = np.asarray(value, np.float32)
    mask = np.asarray(mask)
    Wq = np.asarray(Wq, np.float32)
    Wk = np.asarray(Wk, np.float32)
    Wv = np.asarray(Wv, np.float32)
    Wo = np.asarray(Wo, np.float32)
    bq = np.asarray(bq, np.float32)
    bk = np.asarray(bk, np.float32)
    bv = np.asarray(bv, np.float32)

    keep = [np.flatnonzero(~mask[b, 0]) for b in range(B)]
    ktk = max(1, max((len(k) + 127) // 128 for k in keep))
    SKc = 128 * ktk
    ndt = _np_dt()

    in_maps = []
    for c in range(NCORES):
        b, g = c // G, c % G
        cs = slice(g * DG, (g + 1) * DG)
        idx = keep[b]
        nk = len(idx)
        xkc = np.zeros((D, SKc), np.float32)
        xvc = np.zeros((D, SKc), np.float32)
        xkc[:, :nk] = key[b].T[:, idx]
        xvc[:, :nk] = value[b].T[:, idx]
        mbias = np.full(SKc, MASK_NEG, np.float32)
        mbias[:nk] = 0.0

        wv_aug = np.zeros((D + 1, VW), np.float32)
        for j in range(HPG):
            src = slice(g * DG + j * DK, g * DG + (j + 1) * DK)
            wv_aug[:D, j * 65 : j * 65 + 64] = Wv[:, src]
            wv_aug[D, j * 65 : j * 65 + 64] = bv[src]
            wv_aug[D, j * 65 + 64] = 1.0

        in_maps.append(
            {
                "xq": np.ascontiguousarray(query[b].T).astype(ndt),
                "xk": xkc.astype(ndt),
                "xv": xvc.astype(ndt),
                "wq": np.ascontiguousarray(Wq[:, cs]).astype(ndt),
                "wk": np.ascontiguousarray(Wk[:, cs]).astype(ndt),
                "wv": wv_aug.astype(ndt),
                "wo": np.ascontiguousarray(Wo[cs, :]).astype(ndt),
                "bq": np.ascontiguousarray(bq[cs].reshape(NT, 128).T),
                "bk": np.ascontiguousarray(bk[cs].reshape(NT, 128).T),
                "mb": np.ascontiguousarray(mbias.reshape(ktk, 128).T),
                "cst": _const_rows(),
                "cstc": np.ones((1, 128), np.float32).astype(ndt),
            }
        )
    return in_maps, ktk


def combine_outputs(results, mask, bo):
    mask = np.asarray(mask)
    bo = np.asarray(bo, np.float32)
    out = np.zeros((B, S, D), np.float32)
    for c in range(NCORES):
        out[c // G] += np.asarray(results[c]["out"], np.float32)
    for b in range(B):
        if mask[b, 0].all():
            # reference: fully-masked rows produce zero context
            out[b] = 0.0
    out += bo[None, None, :]
    return out


_NC_CACHE = {}


def kernel(query, key, value, mask, Wq, bq, Wk, bk, Wv, bv, Wo, bo):
    from concourse.bass_utils import run_bass_kernel_spmd

    in_maps, ktk = make_in_maps(
        query, key, value, mask, Wq, bq, Wk, bk, Wv, bv, Wo, bo
    )
    nc = _NC_CACHE.get((KDT, ktk))
    if nc is None:
        nc = _NC_CACHE[(KDT, ktk)] = build_bass(ktk)
    res = run_bass_kernel_spmd(nc, in_maps, list(range(NCORES))).results
    return combine_outputs(res, mask, bo)


# revision 18
# speedup vs baseline: 1.2624x; 1.2624x over previous
"""Multi-head attention (B=2, S=2048, D=1024, H=16) on 8 NeuronCores.

Sharding: core c -> (batch b = c // 4, head-group g = c % 4). Each core
computes 4 heads of one batch plus the partial output projection for its
head-group's rows of Wo; the host sums the 4 partials per batch and adds bo.

Key-side compaction: masked key positions are dropped on the host; the
compacted length is padded to a multiple of 128 with zero columns whose
exp-bias (-60) keeps their softmax weight at ~0.

v2 structural change vs v1: single fused emission stream. The projection
matmuls (Q chunks 1-3), V-projection, output-projection m-tiles and the
normalization broadcasts are interleaved INTO the attention unit loop as PE
filler, so the Tensor engine streams continuously while the ACT engine
(which carries the exp() pass, ~1.05us per [128,1024] score tile — the
serial floor of the kernel) stays saturated from ~15us onward instead of
idling through two serial projection phases.

Engine placement: exp + K/Q0 bias-evacs on ACT; V/Q-filler/ctx-norm/output
evacuations on DVE; reciprocal via reciprocal_approx_fast (fp32 bit trick,
~5x the iterative divide) + a 1-row f16 cast; normalization broadcast runs
as f16 K=1 matmuls (f32r moving operands run the 4-pass fp32 HIGH mode on
real HW). Output partials leave in bf16 on the GpSimd DMA queue.
"""

import os
from collections import defaultdict
from contextlib import ExitStack

import numpy as np

import concourse.bacc as bacc
import concourse.mybir as mybir
import concourse.tile as tile

F32 = mybir.dt.float32
F32R = mybir.dt.float32r
BF16 = mybir.dt.bfloat16
F16 = mybir.dt.float16
FP8 = mybir.dt.float8e4
AF = mybir.ActivationFunctionType

B, S, D = 2, 2048, 1024
H, DK = 16, 64
G = 4                    # head-groups (tensor parallel)
HPG = H // G             # 4 heads per group
DG = HPG * DK            # 256 head dims per group
NCORES = 8
MASK_NEG = -60.0         # additive post-scale bias for padded key positions
SCALE = 0.125            # 1/sqrt(dk)

KT_D = D // 128          # 8 contraction tiles for projections
NT = DG // 128           # 2 partition-tiles of qT/kT/cT (one head-pair each)
QC = 512                 # q chunk (matmul moving dim)
NQC = S // QC            # 4
VW = HPG * (DK + 1)      # 260: V width incl. per-head ones column

KDT = os.environ.get("KDT", "bf16")


def _dt():
    return BF16 if KDT == "bf16" else F32R


def _np_dt():
    import ml_dtypes

    return ml_dtypes.bfloat16 if KDT == "bf16" else np.float32


def build_bass(ktk):
    """Build the SPMD program for `ktk` 128-wide key tiles (SK = 128*ktk)."""
    SK = 128 * ktk
    kchunks = [(n0, min(QC, SK - n0)) for n0 in range(0, SK, QC)]
    cdt = _dt()
    NU = NQC * NT        # 8 attention units

    nc = bacc.Bacc(None, target_bir_lowering=False, debug=False)

    xq = nc.dram_tensor("xq", [D, S], cdt, kind="ExternalInput")
    xk = nc.dram_tensor("xk", [D, SK], cdt, kind="ExternalInput")
    xv = nc.dram_tensor("xv", [D, SK], cdt, kind="ExternalInput")
    wq = nc.dram_tensor("wq", [D, DG], cdt, kind="ExternalInput")
    wk = nc.dram_tensor("wk", [D, DG], cdt, kind="ExternalInput")
    wv = nc.dram_tensor("wv", [D + 1, VW], cdt, kind="ExternalInput")
    wo = nc.dram_tensor("wo", [DG, D], cdt, kind="ExternalInput")
    bq = nc.dram_tensor("bq", [128, NT], F32, kind="ExternalInput")
    bk = nc.dram_tensor("bk", [128, NT], F32, kind="ExternalInput")
    mb = nc.dram_tensor("mb", [128, ktk], F32, kind="ExternalInput")
    cst = nc.dram_tensor("cst", [3, 128], F16, kind="ExternalInput")
    cstc = nc.dram_tensor("cstc", [1, 128], cdt, kind="ExternalInput")
    out = nc.dram_tensor("out", [S, D], BF16, kind="ExternalOutput")

    with tile.TileContext(nc) as tc, ExitStack() as ctx:
        consts = ctx.enter_context(tc.tile_pool(name="consts", bufs=1))
        resid = ctx.enter_context(tc.tile_pool(name="resid", bufs=1))
        stream = ctx.enter_context(tc.tile_pool(name="stream", bufs=1))
        ptp = ctx.enter_context(tc.tile_pool(name="ptp", bufs=6))
        smalls = ctx.enter_context(tc.tile_pool(name="smalls", bufs=3))
        obp = ctx.enter_context(tc.tile_pool(name="obp", bufs=3))

        # ---------------- weights / consts / input streams ----------------
        # DMA emission order == consumption order; everything rides the Sync
        # HWDGE queue except output tiles (GpSimd queue).
        wk_s = consts.tile([128, KT_D, DG], cdt, tag="wk_s", name="wk_s")
        nc.sync.dma_start(out=wk_s, in_=wk[:].rearrange("(kt p) n -> p kt n", p=128))
        bk_s = consts.tile([128, NT], F32, tag="bk_s", name="bk_s")
        nc.gpsimd.dma_start(out=bk_s, in_=bk[:])
        mb_s = consts.tile([128, ktk], F32, tag="mb_s", name="mb_s")
        nc.gpsimd.dma_start(out=mb_s, in_=mb[:])
        ones1 = consts.tile([1, 128], cdt, tag="ones1", name="ones1")
        nc.gpsimd.dma_start(out=ones1, in_=cstc[0:1, :])
        onesA = consts.tile([1, 128], F16, tag="onesA", name="onesA")
        nc.gpsimd.dma_start(out=onesA, in_=cst[1:2, :])
        onesB = consts.tile([1, 128], F16, tag="onesB", name="onesB")
        nc.gpsimd.dma_start(out=onesB, in_=cst[2:3, :])

        # xk in 3 kt-segments so K-projection can chase the DMA
        xk_t = stream.tile([128, KT_D, SK], cdt, tag="xk", name="xk_t")
        for s0 in range(0, KT_D, 3):
            s1 = min(s0 + 3, KT_D)
            nc.sync.dma_start(
                out=xk_t[:, s0:s1, :],
                in_=xk[s0 * 128 : s1 * 128, :].rearrange(
                    "(kt p) n -> p kt n", p=128
                ),
            )

        # wq/xq0/wv/xq1/wo ride the Scalar HWDGE queue, overlapping the Sync
        # queue's wk+xk+xv stream (ACT is idle during the prologue).
        wq_s = consts.tile([128, KT_D, DG], cdt, tag="wq_s", name="wq_s")
        nc.scalar.dma_start(
            out=wq_s, in_=wq[:].rearrange("(kt p) n -> p kt n", p=128)
        )
        bq_s = consts.tile([128, NT], F32, tag="bq_s", name="bq_s")
        nc.gpsimd.dma_start(out=bq_s, in_=bq[:])

        xq_tiles = []

        def load_xq(qc, eng=None):
            t_ = stream.tile(
                [128, KT_D, QC], cdt, tag="xq", bufs=3, name=f"xq{qc}"
            )
            (eng or nc.sync).dma_start(
                out=t_,
                in_=xq[:, qc * QC : (qc + 1) * QC].rearrange(
                    "(kt p) c -> p kt c", p=128
                ),
            )
            xq_tiles.append(t_)

        load_xq(0, eng=nc.scalar)

        wv_s = consts.tile([128, KT_D, VW], cdt, tag="wv_s", name="wv_s")
        nc.scalar.dma_start(
            out=wv_s, in_=wv[0:D, :].rearrange("(kt p) n -> p kt n", p=128)
        )
        wv_b = consts.tile([1, VW], cdt, tag="wv_b", name="wv_b")
        nc.gpsimd.dma_start(out=wv_b, in_=wv[D : D + 1, :])

        xv_t = stream.tile([128, KT_D, SK], cdt, tag="xv", name="xv_t")

        def load_xv(m):
            nc.sync.dma_start(
                out=xv_t[:, :, m * 128 : (m + 1) * 128],
                in_=xv[:, m * 128 : (m + 1) * 128].rearrange(
                    "(kt p) c -> p kt c", p=128
                ),
            )

        load_xq(1, eng=nc.scalar)
        for m in range(ktk):
            load_xv(m)
        wo_s = consts.tile([128, NT, D], cdt, tag="wo_s", name="wo_s")
        nc.scalar.dma_start(
            out=wo_s, in_=wo[:].rearrange("(t p) n -> p t n", p=128)
        )
        load_xq(2)
        load_xq(3)

        # ---------------- resident activations ----------------
        qT = [resid.tile([128, S], cdt, tag=f"qT{t}", name=f"qT{t}") for t in range(NT)]
        kT = [resid.tile([128, SK], cdt, tag=f"kT{t}", name=f"kT{t}") for t in range(NT)]
        v_s = resid.tile([128, ktk, VW], cdt, tag="v_s", name="v_s")
        cT = [resid.tile([128, S], cdt, tag=f"cT{t}", name=f"cT{t}") for t in range(NT)]

        with tc.tile_pool(name="pa", bufs=1, space="PSUM") as pa:
            # PSUM budget: tags ps([128,1024]f32, 2 banks) x2 bufs + pcA/pcB
            # ([65,512]f32, 1 bank) x2 bufs each = 8 banks. Projections,
            # pbc and pom all borrow slots from the "ps" rotation.
            def kproj_chunk(t, ci, on_act=True):
                n0, w = kchunks[ci]
                psk = pa.tile([128, QC], F32, tag="ps", bufs=2, name="psk")
                for kt in range(KT_D):
                    nc.tensor.matmul(
                        psk[:, 0:w],
                        lhsT=wk_s[:, kt, t * 128 : (t + 1) * 128],
                        rhs=xk_t[:, kt, n0 : n0 + w],
                        start=(kt == 0),
                        stop=(kt == KT_D - 1),
                    )
                if on_act:
                    nc.scalar.activation(
                        out=kT[t][:, n0 : n0 + w],
                        in_=psk[:, 0:w],
                        func=AF.Identity,
                        bias=bk_s[:, t : t + 1],
                        scale=1.0,
                    )
                else:
                    nc.vector.tensor_scalar_add(
                        kT[t][:, n0 : n0 + w], psk[:, 0:w], bk_s[:, t : t + 1]
                    )

            def qproj(qc, t, on_act=False):
                xt = xq_tiles[qc]
                psq = pa.tile([128, QC], F32, tag="ps", bufs=2, name="psq")
                for kt in range(KT_D):
                    nc.tensor.matmul(
                        psq[:],
                        lhsT=wq_s[:, kt, t * 128 : (t + 1) * 128],
                        rhs=xt[:, kt, :],
                        start=(kt == 0),
                        stop=(kt == KT_D - 1),
                    )
                qsl = slice(qc * QC, (qc + 1) * QC)
                if on_act:
                    nc.scalar.activation(
                        out=qT[t][:, qsl],
                        in_=psq[:],
                        func=AF.Identity,
                        bias=bq_s[:, t : t + 1],
                        scale=1.0,
                    )
                else:
                    nc.vector.tensor_scalar_add(
                        qT[t][:, qsl], psq[:], bq_s[:, t : t + 1]
                    )

            def vproj(m):
                pvm = pa.tile([128, VW], F32, tag="ps", bufs=2, name="pvm")
                for kt in range(KT_D):
                    nc.tensor.matmul(
                        pvm[:],
                        lhsT=xv_t[:, kt, m * 128 : (m + 1) * 128],
                        rhs=wv_s[:, kt, :],
                        start=(kt == 0),
                        stop=False,
                    )
                nc.tensor.matmul(
                    pvm[:], lhsT=ones1[:], rhs=wv_b[:], start=False, stop=True
                )
                nc.vector.tensor_copy(v_s[:, m, :], pvm[:])

            def emit_final_m(m, on_act=False):
                pom = pa.tile([128, D], F32, tag="ps", bufs=2, name="pom")
                for oc in range(2):
                    for t in range(NT):
                        nc.tensor.matmul(
                            pom[:, oc * 512 : (oc + 1) * 512],
                            lhsT=cT[t][:, m * 128 : (m + 1) * 128],
                            rhs=wo_s[:, t, oc * 512 : (oc + 1) * 512],
                            start=(t == 0),
                            stop=(t == NT - 1),
                        )
                ob = obp.tile([128, D], BF16, tag="ob", name="ob")
                if on_act:
                    nc.scalar.copy(ob[:], pom[:])
                else:
                    nc.vector.tensor_copy(ob[:], pom[:])
                nc.gpsimd.dma_start(out=out[m * 128 : (m + 1) * 128, :], in_=ob[:])

            def emit_recips(pcA, pcB, act_stage=False):
                # reciprocal_approx_fast only works at partition 0, so the
                # denominator rows (row 64 of the ctx PSUMs) are staged there
                # first; the f16 cast feeds the full-rate PE broadcast. The
                # last unit stages via ACT (idle after the final exp) so the
                # tail chain doesn't queue behind DVE evacuations.
                den = smalls.tile([1, 2 * QC], F32, tag="den", name="den")
                if act_stage:
                    nc.scalar.copy(den[0:1, 0:QC], pcA[64:65, :])
                    nc.scalar.copy(den[0:1, QC : 2 * QC], pcB[64:65, :])
                else:
                    nc.vector.tensor_copy(den[0:1, 0:QC], pcA[64:65, :])
                    nc.vector.tensor_copy(den[0:1, QC : 2 * QC], pcB[64:65, :])
                rec32 = smalls.tile([1, 2 * QC], F32, tag="rec32", name="rec32")
                nc.vector.reciprocal_approx_fast(rec32[:], den[:])
                rec = smalls.tile([1, 2 * QC], F16, tag="rec", name="rec")
                with nc.allow_low_precision(reason="f16 feed for PE broadcast"):
                    nc.vector.tensor_copy(rec[:], rec32[:])
                return rec

            def emit_norm(qc, p, pcA, pcB, rec):
                qsl = slice(qc * QC, (qc + 1) * QC)
                pbc = pa.tile([128, QC], F32, tag="ps", bufs=2, name="pbc")
                nc.tensor.matmul(
                    pbc[:],
                    lhsT=onesA[:],
                    rhs=rec[0:1, 0:QC],
                    start=True,
                    stop=False,
                )
                nc.tensor.matmul(
                    pbc[:],
                    lhsT=onesB[:],
                    rhs=rec[0:1, QC : 2 * QC],
                    start=False,
                    stop=True,
                )
                bcs = smalls.tile([128, QC], F32, tag="bcs", name="bcs")
                nc.vector.tensor_copy(bcs[:], pbc[:])
                nc.vector.tensor_mul(cT[p][0:64, qsl], pcA[0:64, :], bcs[0:64, :])
                nc.vector.tensor_mul(
                    cT[p][64:128, qsl], pcB[0:64, :], bcs[64:128, :]
                )

            # ---------------- prologue ----------------
            for ci in range(len(kchunks)):
                kproj_chunk(0, ci)
            qproj(0, 0, on_act=True)

            # ---------------- fused attention + filler schedule ----------
            units = [(qc, p) for qc in range(NQC) for p in range(NT)]
            unit_fillers = defaultdict(list)
            tail_fillers = []
            # Unit 0 carries K-pair-1 and Q(0,1); its V-projection m-tiles
            # ride inline one iteration ahead of the consuming ctx matmuls.
            nkc = len(kchunks)
            for ci in range(nkc):
                unit_fillers[0].append(
                    lambda ci=ci: kproj_chunk(1, ci, on_act=False)
                )
            unit_fillers[0].append(lambda: qproj(0, 1))
            # Q projections for chunks 1..3, at least one unit ahead.
            qlist = [(qc, t) for qc in range(1, NQC) for t in range(NT)]
            slots = [1, 1, 2, 3, 4, 5]
            for (qc, t), u in zip(qlist, slots):
                unit_fillers[u].append(lambda qc=qc, t=t: qproj(qc, t))
            # Output projection m-tiles: legal from unit 2qc+2 (norm of the
            # second pair of qc is emitted at that unit's kt==2).
            for qc in range(NQC):
                base = 2 * qc + 2
                for j in range(4):
                    m = qc * 4 + j
                    u = base + j // 2 + (1 if qc < 2 else 0)
                    if u < NU and qc < 3 and m != 11:
                        unit_fillers[u].append(
                            lambda m=m, a=(u >= 6): emit_final_m(m, on_act=a)
                        )

            norm_kt = min(2, ktk - 1)
            pend = {}
            normed = set()
            state = {}
            seq = [(i, kt) for i in range(NU) for kt in range(ktk)]
            fqs = {i: list(unit_fillers.get(i, ())) for i in range(NU)}

            def emit_scores(i, kt):
                qc, p = units[i]
                qsl = slice(qc * QC, (qc + 1) * QC)
                ksl = slice(kt * 128, (kt + 1) * 128)
                ps = pa.tile([128, 2 * QC], F32, tag="ps", bufs=2, name="ps")
                nc.tensor.matmul(
                    ps[:, 0:QC],
                    lhsT=kT[p][0:64, ksl],
                    rhs=qT[p][0:64, qsl],
                    start=True,
                    stop=True,
                )
                nc.tensor.matmul(
                    ps[:, QC : 2 * QC],
                    lhsT=kT[p][64:128, ksl],
                    rhs=qT[p][64:128, qsl],
                    start=True,
                    stop=True,
                )
                pt = ptp.tile([128, 2 * QC], cdt, tag="pt", name="pt")
                nc.scalar.activation(
                    out=pt[:],
                    in_=ps[:],
                    func=AF.Exp,
                    bias=mb_s[:, kt : kt + 1],
                    scale=SCALE,
                )
                state[i]["pts"].append(pt)

            def emit_ctx(i, ct):
                qc, p = units[i]
                st = state[i]
                if ct == 0:
                    st["pcA"] = pa.tile([65, QC], F32, tag="pcA", bufs=2, name="pcA")
                    st["pcB"] = pa.tile([65, QC], F32, tag="pcB", bufs=2, name="pcB")
                hA, hB = 2 * p, 2 * p + 1
                ptc = st["pts"][ct]
                nc.tensor.matmul(
                    st["pcA"][:],
                    lhsT=v_s[:, ct, hA * 65 : (hA + 1) * 65],
                    rhs=ptc[:, 0:QC],
                    start=(ct == 0),
                    stop=(ct == ktk - 1),
                )
                nc.tensor.matmul(
                    st["pcB"][:],
                    lhsT=v_s[:, ct, hB * 65 : (hB + 1) * 65],
                    rhs=ptc[:, QC : 2 * QC],
                    start=(ct == 0),
                    stop=(ct == ktk - 1),
                )
                if ct == ktk - 1:
                    pend[i] = (
                        st["pcA"],
                        st["pcB"],
                        emit_recips(st["pcA"], st["pcB"], act_stage=(i == NU - 1)),
                    )

            for g, (i, kt) in enumerate(seq):
                if kt == 0:
                    state[i] = {"pts": []}
                emit_scores(i, kt)
                fill_kts = (1, 3, 5, 7) if i == 0 else (3, 5, 7)
                if kt in fill_kts and fqs[i]:
                    fqs[i].pop(0)()
                if kt == ktk - 1:
                    while fqs[i]:  # tiny-ktk fallback: drain before unit ends
                        fqs[i].pop(0)()
                if i == 0 and kt < ktk:
                    vproj(kt)
                if g >= 1:
                    pi, pkt = seq[g - 1]
                    emit_ctx(pi, pkt)
                if i >= 1 and i - 1 in pend and i - 1 not in normed and kt >= norm_kt:
                    normed.add(i - 1)
                    pqc, pp_ = units[i - 1]
                    emit_norm(pqc, pp_, *pend[i - 1])
            emit_ctx(*seq[-1])

            lqc, lp = units[-1]
            emit_final_m(11)  # reserved bridge tile: keeps the PE warm
            emit_norm(lqc, lp, *pend[NU - 1])
            for qc3 in range(4):
                emit_final_m(12 + qc3, on_act=True)

    nc.compile()
    return nc


def _const_rows():
    cst = np.zeros((3, 128), np.float16)
    cst[0, :] = 1.0
    cst[1, 0:64] = 1.0
    cst[2, 64:128] = 1.0
    return cst


def make_in_maps(query, key, value, mask, Wq, bq, Wk, bk, Wv, bv, Wo, bo):
    """Returns (in_maps, ktk). Key positions with mask=True are dropped."""
    query = np.asarray(query, np.float32)
    key = np.asarray(key, np.float32)
    value = np.asarray(value, np.float32)
    mask = np.asarray(mask)
    Wq = np.asarray(Wq, np.float32)
    Wk = np.asarray(Wk, np.float32)
    Wv = np.asarray(Wv, np.float32)
    Wo = np.asarray(Wo, np.float32)
    bq = np.asarray(bq, np.float32)
    bk = np.asarray(bk, np.float32)
    bv = np.asarray(bv, np.float32)

    keep = [np.flatnonzero(~mask[b, 0]) for b in range(B)]
    ktk = max(1, max((len(k) + 127) // 128 for k in keep))
    SKc = 128 * ktk
    ndt = _np_dt()

    in_maps = []
    for c in range(NCORES):
        b, g = c // G, c % G
        cs = slice(g * DG, (g + 1) * DG)
        idx = keep[b]
        nk = len(idx)
        xkc = np.zeros((D, SKc), np.float32)
        xvc = np.zeros((D, SKc), np.float32)
        xkc[:, :nk] = key[b].T[:, idx]
        xvc[:, :nk] = value[b].T[:, idx]
        mbias = np.full(SKc, MASK_NEG, np.float32)
        mbias[:nk] = 0.0

        wv_aug = np.zeros((D + 1, VW), np.float32)
        for j in range(HPG):
            src = slice(g * DG + j * DK, g * DG + (j + 1) * DK)
            wv_aug[:D, j * 65 : j * 65 + 64] = Wv[:, src]
            wv_aug[D, j * 65 : j * 65 + 64] = bv[src]
            wv_aug[D, j * 65 + 64] = 1.0

        in_maps.append(
            {
                "xq": np.ascontiguousarray(query[b].T).astype(ndt),
                "xk": xkc.astype(ndt),
                "xv": xvc.astype(ndt),
                "wq": np.ascontiguousarray(Wq[:, cs]).astype(ndt),
                "wk": np.ascontiguousarray(Wk[:, cs]).astype(ndt),
                "wv": wv_aug.astype(ndt),
                "wo": np.ascontiguousarray(Wo[cs, :]).astype(ndt),
                "bq": np.ascontiguousarray(bq[cs].reshape(NT, 128).T),
                "bk": np.ascontiguousarray(bk[cs].reshape(NT, 128).T),
                "mb": np.ascontiguousarray(mbias.reshape(ktk, 128).T),
                "cst": _const_rows(),
                "cstc": np.ones((1, 128), np.float32).astype(ndt),
            }
        )
    return in_maps, ktk


def combine_outputs(results, mask, bo):
    mask = np.asarray(mask)
    bo = np.asarray(bo, np.float32)
    out = np.zeros((B, S, D), np.float32)
    for c in range(NCORES):
        out[c // G] += np.asarray(results[c]["out"], np.float32)
    for b in range(B):
        if mask[b, 0].all():
            # reference: fully-masked rows produce zero context
            out[b] = 0.0
    out += bo[None, None, :]
    return out


_NC_CACHE = {}


def kernel(query, key, value, mask, Wq, bq, Wk, bk, Wv, bv, Wo, bo):
    from concourse.bass_utils import run_bass_kernel_spmd

    in_maps, ktk = make_in_maps(
        query, key, value, mask, Wq, bq, Wk, bk, Wv, bv, Wo, bo
    )
    nc = _NC_CACHE.get((KDT, ktk))
    if nc is None:
        nc = _NC_CACHE[(KDT, ktk)] = build_bass(ktk)
    res = run_bass_kernel_spmd(nc, in_maps, list(range(NCORES))).results
    return combine_outputs(res, mask, bo)
